# revision 3
# baseline (speedup 1.0000x reference)
"""Trainium2 Bass kernel for nn_BrainLayer (echo-state reservoir network).

Time-parallel scheme (zero collectives):
  The leaky ESN forgets its initial condition at ~0.79x/step, so each of
  the 8 cores computes an independent 64-step output segment, preceded by
  a 32-step burn-in anchored at the true initial state (cores 1-7; core 0
  starts exactly at t=0 and needs none).  Measured end-to-end error of
  this approximation in fp16 is ~6e-4, far inside the 2e-2 gate.

  Every core runs the identical 96-step full-state recurrence (SPMD);
  only its x time-slice differs.  The host keeps outs[0:64) from core 0
  and outs[32:96) from cores 1-7.

Numerics: gamma is folded into W_rec via the substitution u = r/gamma
(u' = (1-gamma)*u + tanh((gamma*W_rec)u + W_in x + b)), so the per-step
update is one fused scalar_tensor_tensor on DVE.  Weights/state/x are
fp16, PSUM accumulation f32, tanh on the Act engine straight from PSUM.
The host multiplies the gathered outputs by gamma.

Per step: 16 m-groups x (W_in + bias-ones + 16 W_rec) matmuls (m-outer,
accumulation groups contiguous), split in two halves so tanh+blend of
half A overlaps the matmuls of half B and the next step's k-loop starts
on half A before half B lands.
"""

import numpy as np

import concourse.bacc as bacc
import concourse.tile as tile
import concourse.mybir as mybir
from concourse.bass_utils import run_bass_kernel_spmd

N = 2048          # reservoir
F = 128           # features
B = 32            # batch
T = 512           # time steps
GAMMA = 0.95
N_CORES = 8
SEG = T // N_CORES            # 64 output steps per core
BURN = 24                     # burn-in steps (error ~5e-4 in f32)
S = SEG + BURN                # 88 program steps per core
MF = N // 128                 # 16 m-groups
KC = N // 128                 # 16 state k-chunks

F16 = mybir.dt.float16
F32 = mybir.dt.float32

_cache = {}


def _build():
    nc = bacc.Bacc("TRN2", target_bir_lowering=False, debug=False,
                   num_devices=N_CORES)

    w_dram = nc.dram_tensor("w", [128, MF * (1 + KC) * 128], F16,
                            kind="ExternalInput")
    xt_dram = nc.dram_tensor("xt", [128, S * B], F16, kind="ExternalInput")
    biasv_dram = nc.dram_tensor("biasv", [128, MF], F32,
                                kind="ExternalInput")
    u0_dram = nc.dram_tensor("u0", [128, KC * B], F16, kind="ExternalInput")
    outs_dram = nc.dram_tensor("outs", [S, 128, KC * B], F16,
                               kind="ExternalOutput")

    with tile.TileContext(nc) as tc:
        with tc.tile_pool(name="cst", bufs=1) as cst, \
             tc.tile_pool(name="sb", bufs=2) as sb, \
             tc.tile_pool(name="ps", bufs=2, space="PSUM") as pp:

            w_sb = cst.tile([128, MF * (1 + KC) * 128], F16)
            nc.sync.dma_start(w_sb[:], w_dram[:])
            xt_sb = cst.tile([128, S * B], F16)
            nc.sync.dma_start(xt_sb[:], xt_dram[:])
            biasv_sb = cst.tile([128, MF], F32)
            nc.sync.dma_start(biasv_sb[:], biasv_dram[:])

            u = sb.tile([128, KC * B], F16, tag="u", name="u_init")
            nc.sync.dma_start(u[:], u0_dram[:])

            def wtile(m, kk):
                i = (m * (1 + KC) + kk) * 128
                return w_sb[:, i:i + 128]

            # Per-m-group phases: 17 matmuls (W_in + 16 W_rec k-chunks) into
            # an m-private PSUM bank, then tanh (bias folded into the Act
            # bias AP) and the leaky blend on that group's 32 columns.  Each
            # of the 8 PSUM banks is owned by m and m+8 (bufs=1); the tanh
            # read of bank b never overlaps matmul writes to bank b because
            # group m+8 starts 7 groups (~4us) after ACT-m finished.
            for t in range(S):
                th = sb.tile([128, MF * B], F16, tag="th", name=f"th{t}")
                u_new = sb.tile([128, KC * B], F16, tag="u", name=f"u{t + 1}")
                for m in range(MF):
                    psm = pp.tile([128, 512], F32, tag=f"ps{m % 8}",
                                  name=f"ps{m}_{t}", bufs=1)
                    o = psm[:, :B]
                    nc.tensor.matmul(o, wtile(m, 0),
                                     xt_sb[:, t * B:(t + 1) * B],
                                     start=True, stop=False)
                    for kk in range(KC):
                        nc.tensor.matmul(o, wtile(m, 1 + kk),
                                         u[:, kk * B:(kk + 1) * B],
                                         start=False, stop=(kk == KC - 1))
                    ms = slice(m * B, (m + 1) * B)
                    nc.scalar.activation(th[:, ms], o,
                                         mybir.ActivationFunctionType.Tanh,
                                         bias=biasv_sb[:, m:m + 1])
                    nc.vector.scalar_tensor_tensor(
                        u_new[:, ms], u[:, ms], 1.0 - GAMMA, th[:, ms],
                        op0=mybir.AluOpType.mult, op1=mybir.AluOpType.add)
                nc.sync.dma_start(outs_dram[t], u_new[:])
                u = u_new
    nc.compile()
    return nc


def _prep_inputs(x, input_weights, recurrent_weights, bias, reservoir_start,
                 in_cor):
    eye = np.eye(N, dtype=np.float32)
    if np.array_equal(in_cor, eye):
        w_in_eff = input_weights.astype(np.float32)
    else:
        w_in_eff = (in_cor.astype(np.float32)
                    @ input_weights.astype(np.float32))
    w_rec_eff = np.float32(GAMMA) * recurrent_weights.astype(np.float32)

    wt = np.empty((128, MF * (1 + KC) * 128), dtype=np.float32)
    for m in range(MF):
        base = m * (1 + KC) * 128
        wt[:, base:base + 128] = w_in_eff[128 * m:128 * (m + 1), :].T
        for kk in range(KC):
            i = base + (1 + kk) * 128
            wt[:, i:i + 128] = w_rec_eff[128 * m:128 * (m + 1),
                                         128 * kk:128 * (kk + 1)].T
    wt = wt.astype(np.float16)

    # biasv[p, m] = bias[128*m + p]
    biasv = np.ascontiguousarray(
        bias.astype(np.float32).reshape(MF, 128).T)

    u0_vec = (reservoir_start.astype(np.float32) / np.float32(GAMMA))
    u0 = np.empty((128, KC * B), dtype=np.float32)
    for kk in range(KC):
        u0[:, kk * B:(kk + 1) * B] = np.repeat(
            u0_vec[128 * kk:128 * (kk + 1), None], B, axis=1)
    u0 = u0.astype(np.float16)

    x16 = x.astype(np.float16)
    in_maps = []
    for c in range(N_CORES):
        t0 = max(0, SEG * c - BURN)
        # xt[f, j*B + b] = x[b, t0 + j, f]
        xt = np.ascontiguousarray(
            x16[:, t0:t0 + S, :].transpose(2, 1, 0).reshape(F, S * B))
        in_maps.append({"w": wt, "xt": xt, "biasv": biasv, "u0": u0})
    return in_maps


def _assemble(results, out_cor):
    full = np.empty((B, T, N), dtype=np.float32)
    for c in range(N_CORES):
        pick = 0 if c == 0 else BURN
        o = results[c]["outs"][pick:pick + SEG]      # [SEG, 128, KC*B] f16
        o = o.reshape(SEG, 128, KC, B)
        # full[b, SEG*c + j, 128*kc + p] = gamma * o[j, p, kc, b]
        full[:, SEG * c:SEG * (c + 1), :] = (
            o.transpose(3, 0, 2, 1).reshape(B, SEG, N).astype(np.float32))
    full *= np.float32(GAMMA)
    eye = np.eye(N, dtype=np.float32)
    if not np.array_equal(out_cor, eye):
        full = full @ out_cor.astype(np.float32).T
    return full


def kernel(x, input_weights, recurrent_weights, bias, reservoir_start,
           in_cor, out_cor, _trace=False):
    x = np.asarray(x, dtype=np.float32)
    assert x.shape == (B, T, F)
    in_maps = _prep_inputs(x, np.asarray(input_weights),
                           np.asarray(recurrent_weights), np.asarray(bias),
                           np.asarray(reservoir_start), np.asarray(in_cor))
    if "nc" not in _cache:
        _cache["nc"] = _build()
    nc = _cache["nc"]
    res = run_bass_kernel_spmd(nc, in_maps, core_ids=list(range(N_CORES)),
                               trace=_trace)
    out = _assemble(res.results, np.asarray(out_cor))
    kernel.last_exec_time_ns = res.exec_time_ns
    return out


kernel.last_exec_time_ns = None


# revision 4
# speedup vs baseline: 1.2065x; 1.2065x over previous
"""Trainium2 Bass kernel for nn_BrainLayer (echo-state reservoir network).

Time-parallel scheme (zero collectives):
  The leaky ESN forgets its initial condition at ~0.79x/step, so each of
  the 8 cores computes an independent 64-step output segment, preceded by
  a 32-step burn-in anchored at the true initial state (cores 1-7; core 0
  starts exactly at t=0 and needs none).  Measured end-to-end error of
  this approximation in fp16 is ~6e-4, far inside the 2e-2 gate.

  Every core runs the identical 96-step full-state recurrence (SPMD);
  only its x time-slice differs.  The host keeps outs[0:64) from core 0
  and outs[32:96) from cores 1-7.

Numerics: gamma is folded into W_rec via the substitution u = r/gamma
(u' = (1-gamma)*u + tanh((gamma*W_rec)u + W_in x + b)), so the per-step
update is one fused scalar_tensor_tensor on DVE.  Weights/state/x are
fp16, PSUM accumulation f32, tanh on the Act engine straight from PSUM.
The host multiplies the gathered outputs by gamma.

Per step: 16 m-groups x (W_in + bias-ones + 16 W_rec) matmuls (m-outer,
accumulation groups contiguous), split in two halves so tanh+blend of
half A overlaps the matmuls of half B and the next step's k-loop starts
on half A before half B lands.
"""

import numpy as np

import concourse.bacc as bacc
import concourse.tile as tile
import concourse.mybir as mybir
from concourse.bass_utils import run_bass_kernel_spmd

N = 2048          # reservoir
F = 128           # features
B = 32            # batch
T = 512           # time steps
GAMMA = 0.95
N_CORES = 8
BURN = 20                     # burn-in steps (error ~1.2e-3 in f32)
# Unequal split: core 0 needs no burn-in, so it takes a longer segment and
# every core's program shrinks to S steps.  Segment starts: core 0 covers
# [0, S), core c>=1 covers [S + (c-1)*(S-BURN), ...); coverage must reach T.
S = 82                        # 82 + 7*62 = 516 >= 512
SEG_STARTS = [0] + [S + (c - 1) * (S - BURN) for c in range(1, N_CORES)]
SEG_ENDS = SEG_STARTS[1:] + [T]
MF = N // 128                 # 16 m-groups
KC = N // 128                 # 16 state k-chunks

F16 = mybir.dt.float16
F32 = mybir.dt.float32

_cache = {}


def _build():
    nc = bacc.Bacc("TRN2", target_bir_lowering=False, debug=False,
                   num_devices=N_CORES)

    w_dram = nc.dram_tensor("w", [128, MF * (1 + KC) * 128], F16,
                            kind="ExternalInput")
    xt_dram = nc.dram_tensor("xt", [128, S * B], F16, kind="ExternalInput")
    biasv_dram = nc.dram_tensor("biasv", [128, MF], F32,
                                kind="ExternalInput")
    u0_dram = nc.dram_tensor("u0", [128, KC * B], F16, kind="ExternalInput")
    outs_dram = nc.dram_tensor("outs", [S, 128, KC * B], F16,
                               kind="ExternalOutput")

    with tile.TileContext(nc) as tc:
        with tc.tile_pool(name="cst", bufs=1) as cst, \
             tc.tile_pool(name="sb", bufs=2) as sb, \
             tc.tile_pool(name="ps", bufs=2, space="PSUM") as pp:

            w_sb = cst.tile([128, MF * (1 + KC) * 128], F16)
            nc.sync.dma_start(w_sb[:], w_dram[:])
            xt_sb = cst.tile([128, S * B], F16)
            nc.sync.dma_start(xt_sb[:], xt_dram[:])
            biasv_sb = cst.tile([128, MF], F32)
            nc.sync.dma_start(biasv_sb[:], biasv_dram[:])

            u = sb.tile([128, KC * B], F16, tag="u", name="u_init")
            nc.sync.dma_start(u[:], u0_dram[:])

            def wtile(m, kk):
                i = (m * (1 + KC) + kk) * 128
                return w_sb[:, i:i + 128]

            # Per-m-group phases: 17 matmuls (W_in + 16 W_rec k-chunks) into
            # an m-private PSUM bank, then tanh (bias folded into the Act
            # bias AP) and the leaky blend on that group's 32 columns.  Each
            # of the 8 PSUM banks is owned by m and m+8 (bufs=1); the tanh
            # read of bank b never overlaps matmul writes to bank b because
            # group m+8 starts 7 groups (~4us) after ACT-m finished.
            for t in range(S):
                th = sb.tile([128, MF * B], F16, tag="th", name=f"th{t}")
                u_new = sb.tile([128, KC * B], F16, tag="u", name=f"u{t + 1}")
                for m in range(MF):
                    psm = pp.tile([128, 512], F32, tag=f"ps{m % 8}",
                                  name=f"ps{m}_{t}", bufs=1)
                    o = psm[:, :B]
                    nc.tensor.matmul(o, wtile(m, 0),
                                     xt_sb[:, t * B:(t + 1) * B],
                                     start=True, stop=False)
                    for kk in range(KC):
                        nc.tensor.matmul(o, wtile(m, 1 + kk),
                                         u[:, kk * B:(kk + 1) * B],
                                         start=False, stop=(kk == KC - 1))
                    ms = slice(m * B, (m + 1) * B)
                    nc.scalar.activation(th[:, ms], o,
                                         mybir.ActivationFunctionType.Tanh,
                                         bias=biasv_sb[:, m:m + 1])
                    nc.vector.scalar_tensor_tensor(
                        u_new[:, ms], u[:, ms], 1.0 - GAMMA, th[:, ms],
                        op0=mybir.AluOpType.mult, op1=mybir.AluOpType.add)
                nc.sync.dma_start(outs_dram[t], u_new[:])
                u = u_new
    nc.compile()
    return nc


def _prep_inputs(x, input_weights, recurrent_weights, bias, reservoir_start,
                 in_cor):
    eye = np.eye(N, dtype=np.float32)
    if np.array_equal(in_cor, eye):
        w_in_eff = input_weights.astype(np.float32)
    else:
        w_in_eff = (in_cor.astype(np.float32)
                    @ input_weights.astype(np.float32))
    w_rec_eff = np.float32(GAMMA) * recurrent_weights.astype(np.float32)

    wt = np.empty((128, MF * (1 + KC) * 128), dtype=np.float32)
    for m in range(MF):
        base = m * (1 + KC) * 128
        wt[:, base:base + 128] = w_in_eff[128 * m:128 * (m + 1), :].T
        for kk in range(KC):
            i = base + (1 + kk) * 128
            wt[:, i:i + 128] = w_rec_eff[128 * m:128 * (m + 1),
                                         128 * kk:128 * (kk + 1)].T
    wt = wt.astype(np.float16)

    # biasv[p, m] = bias[128*m + p]
    biasv = np.ascontiguousarray(
        bias.astype(np.float32).reshape(MF, 128).T)

    u0_vec = (reservoir_start.astype(np.float32) / np.float32(GAMMA))
    u0 = np.empty((128, KC * B), dtype=np.float32)
    for kk in range(KC):
        u0[:, kk * B:(kk + 1) * B] = np.repeat(
            u0_vec[128 * kk:128 * (kk + 1), None], B, axis=1)
    u0 = u0.astype(np.float16)

    x16 = np.zeros((B, T + S, F), dtype=np.float16)   # zero-pad the tail
    x16[:, :T, :] = x.astype(np.float16)
    in_maps = []
    for c in range(N_CORES):
        t0 = 0 if c == 0 else SEG_STARTS[c] - BURN
        # xt[f, j*B + b] = x[b, t0 + j, f]
        xt = np.ascontiguousarray(
            x16[:, t0:t0 + S, :].transpose(2, 1, 0).reshape(F, S * B))
        in_maps.append({"w": wt, "xt": xt, "biasv": biasv, "u0": u0})
    return in_maps


def _assemble(results, out_cor):
    full = np.empty((B, T, N), dtype=np.float32)
    for c in range(N_CORES):
        pick = 0 if c == 0 else BURN
        seg = SEG_ENDS[c] - SEG_STARTS[c]
        o = results[c]["outs"][pick:pick + seg]      # [seg, 128, KC*B] f16
        o = o.reshape(seg, 128, KC, B)
        # full[b, start + j, 128*kc + p] = gamma * o[j, p, kc, b]
        full[:, SEG_STARTS[c]:SEG_ENDS[c], :] = (
            o.transpose(3, 0, 2, 1).reshape(B, seg, N).astype(np.float32))
    full *= np.float32(GAMMA)
    eye = np.eye(N, dtype=np.float32)
    if not np.array_equal(out_cor, eye):
        full = full @ out_cor.astype(np.float32).T
    return full


def kernel(x, input_weights, recurrent_weights, bias, reservoir_start,
           in_cor, out_cor, _trace=False):
    x = np.asarray(x, dtype=np.float32)
    assert x.shape == (B, T, F)
    in_maps = _prep_inputs(x, np.asarray(input_weights),
                           np.asarray(recurrent_weights), np.asarray(bias),
                           np.asarray(reservoir_start), np.asarray(in_cor))
    if "nc" not in _cache:
        _cache["nc"] = _build()
    nc = _cache["nc"]
    res = run_bass_kernel_spmd(nc, in_maps, core_ids=list(range(N_CORES)),
                               trace=_trace)
    out = _assemble(res.results, np.asarray(out_cor))
    kernel.last_exec_time_ns = res.exec_time_ns
    return out


kernel.last_exec_time_ns = None


# revision 5
# speedup vs baseline: 1.2180x; 1.0095x over previous
"""Trainium2 Bass kernel for nn_BrainLayer (echo-state reservoir network).

Time-parallel scheme (zero collectives):
  The leaky ESN forgets its initial condition at ~0.79x/step, so each of
  the 8 cores computes an independent 64-step output segment, preceded by
  a 32-step burn-in anchored at the true initial state (cores 1-7; core 0
  starts exactly at t=0 and needs none).  Measured end-to-end error of
  this approximation in fp16 is ~6e-4, far inside the 2e-2 gate.

  Every core runs the identical 96-step full-state recurrence (SPMD);
  only its x time-slice differs.  The host keeps outs[0:64) from core 0
  and outs[32:96) from cores 1-7.

Numerics: gamma is folded into W_rec via the substitution u = r/gamma
(u' = (1-gamma)*u + tanh((gamma*W_rec)u + W_in x + b)), so the per-step
update is one fused scalar_tensor_tensor on DVE.  Weights/state/x are
fp16, PSUM accumulation f32, tanh on the Act engine straight from PSUM.
The host multiplies the gathered outputs by gamma.

Per step: 16 m-groups x (W_in + bias-ones + 16 W_rec) matmuls (m-outer,
accumulation groups contiguous), split in two halves so tanh+blend of
half A overlaps the matmuls of half B and the next step's k-loop starts
on half A before half B lands.
"""

import numpy as np

import concourse.bacc as bacc
import concourse.tile as tile
import concourse.mybir as mybir
from concourse.bass_utils import run_bass_kernel_spmd

N = 2048          # reservoir
F = 128           # features
B = 32            # batch
T = 512           # time steps
GAMMA = 0.95
N_CORES = 8
BURN = 20                     # burn-in steps (error ~1.2e-3 in f32)
# Unequal split: core 0 needs no burn-in, so it takes a longer segment and
# every core's program shrinks to S steps.  Segment starts: core 0 covers
# [0, S), core c>=1 covers [S + (c-1)*(S-BURN), ...); coverage must reach T.
S = 82                        # 82 + 7*62 = 516 >= 512
SEG_STARTS = [0] + [S + (c - 1) * (S - BURN) for c in range(1, N_CORES)]
SEG_ENDS = SEG_STARTS[1:] + [T]
MF = N // 128                 # 16 m-groups
KC = N // 128                 # 16 state k-chunks

F16 = mybir.dt.float16
F32 = mybir.dt.float32

_cache = {}


def _build():
    nc = bacc.Bacc("TRN2", target_bir_lowering=False, debug=False,
                   num_devices=N_CORES)

    w_dram = nc.dram_tensor("w", [128, MF * (1 + KC) * 128], F16,
                            kind="ExternalInput")
    xt_dram = nc.dram_tensor("xt", [128, S * B], F16, kind="ExternalInput")
    biasv_dram = nc.dram_tensor("biasv", [128, MF], F32,
                                kind="ExternalInput")
    u0_dram = nc.dram_tensor("u0", [128, KC * B], F16, kind="ExternalInput")
    outs_dram = nc.dram_tensor("outs", [S, 128, KC * B], F16,
                               kind="ExternalOutput")

    with tile.TileContext(nc) as tc:
        with tc.tile_pool(name="cst", bufs=1) as cst, \
             tc.tile_pool(name="sb", bufs=2) as sb, \
             tc.tile_pool(name="ps", bufs=2, space="PSUM") as pp:

            # Small inputs first, then W in 8 chunks (2 m-groups each) so
            # step 0's early m-groups start after ~1/8 of the 8.9MB weight
            # load instead of waiting for all of it.
            xt_sb = cst.tile([128, S * B], F16)
            nc.sync.dma_start(xt_sb[:], xt_dram[:])
            biasv_sb = cst.tile([128, MF], F32)
            nc.sync.dma_start(biasv_sb[:], biasv_dram[:])
            u = sb.tile([128, KC * B], F16, tag="u", name="u_init")
            nc.sync.dma_start(u[:], u0_dram[:])
            w_sb = cst.tile([128, MF * (1 + KC) * 128], F16)
            WCH = MF * (1 + KC) * 128 // 8
            for ch in range(8):
                nc.sync.dma_start(w_sb[:, ch * WCH:(ch + 1) * WCH],
                                  w_dram[:, ch * WCH:(ch + 1) * WCH])

            def wtile(m, kk):
                i = (m * (1 + KC) + kk) * 128
                return w_sb[:, i:i + 128]

            # Per-m-group phases: 17 matmuls (W_in + 16 W_rec k-chunks) into
            # an m-private PSUM bank, then tanh (bias folded into the Act
            # bias AP) and the leaky blend on that group's 32 columns.  Each
            # of the 8 PSUM banks is owned by m and m+8 (bufs=1); the tanh
            # read of bank b never overlaps matmul writes to bank b because
            # group m+8 starts 7 groups (~4us) after ACT-m finished.
            for t in range(S):
                th = sb.tile([128, MF * B], F16, tag="th", name=f"th{t}")
                u_new = sb.tile([128, KC * B], F16, tag="u", name=f"u{t + 1}")
                for m in range(MF):
                    psm = pp.tile([128, 512], F32, tag=f"ps{m % 8}",
                                  name=f"ps{m}_{t}", bufs=1)
                    o = psm[:, :B]
                    nc.tensor.matmul(o, wtile(m, 0),
                                     xt_sb[:, t * B:(t + 1) * B],
                                     start=True, stop=False)
                    for kk in range(KC):
                        nc.tensor.matmul(o, wtile(m, 1 + kk),
                                         u[:, kk * B:(kk + 1) * B],
                                         start=False, stop=(kk == KC - 1))
                    ms = slice(m * B, (m + 1) * B)
                    nc.scalar.activation(th[:, ms], o,
                                         mybir.ActivationFunctionType.Tanh,
                                         bias=biasv_sb[:, m:m + 1])
                    nc.vector.scalar_tensor_tensor(
                        u_new[:, ms], u[:, ms], 1.0 - GAMMA, th[:, ms],
                        op0=mybir.AluOpType.mult, op1=mybir.AluOpType.add)
                nc.sync.dma_start(outs_dram[t], u_new[:])
                u = u_new
    nc.compile()
    return nc


def _prep_inputs(x, input_weights, recurrent_weights, bias, reservoir_start,
                 in_cor):
    eye = np.eye(N, dtype=np.float32)
    if np.array_equal(in_cor, eye):
        w_in_eff = input_weights.astype(np.float32)
    else:
        w_in_eff = (in_cor.astype(np.float32)
                    @ input_weights.astype(np.float32))
    w_rec_eff = np.float32(GAMMA) * recurrent_weights.astype(np.float32)

    wt = np.empty((128, MF * (1 + KC) * 128), dtype=np.float32)
    for m in range(MF):
        base = m * (1 + KC) * 128
        wt[:, base:base + 128] = w_in_eff[128 * m:128 * (m + 1), :].T
        for kk in range(KC):
            i = base + (1 + kk) * 128
            wt[:, i:i + 128] = w_rec_eff[128 * m:128 * (m + 1),
                                         128 * kk:128 * (kk + 1)].T
    wt = wt.astype(np.float16)

    # biasv[p, m] = bias[128*m + p]
    biasv = np.ascontiguousarray(
        bias.astype(np.float32).reshape(MF, 128).T)

    u0_vec = (reservoir_start.astype(np.float32) / np.float32(GAMMA))
    u0 = np.empty((128, KC * B), dtype=np.float32)
    for kk in range(KC):
        u0[:, kk * B:(kk + 1) * B] = np.repeat(
            u0_vec[128 * kk:128 * (kk + 1), None], B, axis=1)
    u0 = u0.astype(np.float16)

    x16 = np.zeros((B, T + S, F), dtype=np.float16)   # zero-pad the tail
    x16[:, :T, :] = x.astype(np.float16)
    in_maps = []
    for c in range(N_CORES):
        t0 = 0 if c == 0 else SEG_STARTS[c] - BURN
        # xt[f, j*B + b] = x[b, t0 + j, f]
        xt = np.ascontiguousarray(
            x16[:, t0:t0 + S, :].transpose(2, 1, 0).reshape(F, S * B))
        in_maps.append({"w": wt, "xt": xt, "biasv": biasv, "u0": u0})
    return in_maps


def _assemble(results, out_cor):
    full = np.empty((B, T, N), dtype=np.float32)
    for c in range(N_CORES):
        pick = 0 if c == 0 else BURN
        seg = SEG_ENDS[c] - SEG_STARTS[c]
        o = results[c]["outs"][pick:pick + seg]      # [seg, 128, KC*B] f16
        o = o.reshape(seg, 128, KC, B)
        # full[b, start + j, 128*kc + p] = gamma * o[j, p, kc, b]
        full[:, SEG_STARTS[c]:SEG_ENDS[c], :] = (
            o.transpose(3, 0, 2, 1).reshape(B, seg, N).astype(np.float32))
    full *= np.float32(GAMMA)
    eye = np.eye(N, dtype=np.float32)
    if not np.array_equal(out_cor, eye):
        full = full @ out_cor.astype(np.float32).T
    return full


def kernel(x, input_weights, recurrent_weights, bias, reservoir_start,
           in_cor, out_cor, _trace=False):
    x = np.asarray(x, dtype=np.float32)
    assert x.shape == (B, T, F)
    in_maps = _prep_inputs(x, np.asarray(input_weights),
                           np.asarray(recurrent_weights), np.asarray(bias),
                           np.asarray(reservoir_start), np.asarray(in_cor))
    if "nc" not in _cache:
        _cache["nc"] = _build()
    nc = _cache["nc"]
    res = run_bass_kernel_spmd(nc, in_maps, core_ids=list(range(N_CORES)),
                               trace=_trace)
    out = _assemble(res.results, np.asarray(out_cor))
    kernel.last_exec_time_ns = res.exec_time_ns
    return out


kernel.last_exec_time_ns = None
